# revision 26
# baseline (speedup 1.0000x reference)
"""Trainium2 Bass kernel for nn_AutoCorrelation (softmax attention).

Problem: queries [4,2048,16,64], keys [4,2048,16,64], values [4,2048,16,64]
  scores = einsum('blhe,bshe->bhls', q, k); attn = softmax(scores/8, -1)
  out = einsum('bhls,bshd->blhd', attn, v)      -> [4, 2048, 16, 64] fp32

Sharding: the 64 (batch, head) pairs are split across 8 NeuronCores, 8
heads per core (core c gets batch c//2, heads 8*(c%2) .. 8*(c%2)+8), one
SPMD NEFF with per-core input slices.  While sharding, the host lays q/k
out E-major ([head, 64 e, 2048 l] per core) so the device needs no input
transposes, and the device emits the output transposed ([head, 64 d,
2048 l]); the host undoes that when assembling the full output.

Per-core kernel: heads processed in pairs A/B; per step (s-tile, 512-wide
L window) two row-tiled QK matmuls (heads on disjoint PE row halves)
write one scoresT PSUM tile [128,1024].  The softmax exp is split across
engines: the Act engine exps columns [0:1024-DVE_COLS); the Vector engine
handles the rest via a Schraudolph construction (tensor_scalar computes
x*log2e*2^23 + 127*2^23 written as int32 -- the bit pattern is 2^i*(1+f))
followed by one custom-DVE instruction (bitwise mask + quadratic in
g=1+f) applying the 2^f/(1+f) correction.  PV accumulates
out'T[d(64)+sums(1), lw] over s-tiles in PSUM with V' = [V | ones].
Window epilogue (spread over later steps so no engine FIFO blocks
cross-engine): evict numerator+sums (DVE), reciprocal of sums (custom
DVE), partition-broadcast of the reciprocal row (gpsimd), multiply (DVE),
one output DMA per head.
"""

from contextlib import ExitStack

import numpy as np

import concourse.bass as bass
import concourse.tile as tile
from concourse import bacc, mybir, bass_utils
from concourse import dve_ops
from concourse.dve_spec import Spec, Src0, Src1, C0, C1, C2, One, Bin, Latch, lower
from concourse.dve_uop import AluOp, DveOpSpec
from concourse.dve_table_gen import dve_ver_for

F32 = mybir.dt.float32
I32 = mybir.dt.int32
BF16 = mybir.dt.bfloat16
AF = mybir.ActivationFunctionType
ALU = mybir.AluOpType

B_, L_, H_, E_ = 4, 2048, 16, 64
NCORES = 8
HPC = (B_ * H_) // NCORES  # heads per core = 8

# --- softmax engine split (columns of each [128,1024] scores tile) ---------
DVE_COLS = 176     # columns exp'd via the DVE Schraudolph path
SCALE = 0.125      # 1/sqrt(E)

LOG2E = float(np.log2(np.e))
A_CONST = LOG2E * (1 << 23) * SCALE  # folds the softmax scale
B_CONST = 127.0 * (1 << 23)
MASK_F32 = float(np.int32(0x007FFFFF).view(np.float32))

# quadratic fit of r(g) = 2^(g-1)/g on [1,2)
_gs = np.linspace(1, 2, 8193)[:-1]
_C2Q, _C1Q, _C0Q = [float(v) for v in np.polyfit(_gs, 2 ** (_gs - 1) / _gs, 2)]

LAST_RESULTS = None
_PROG = None


def _register_exp2_corr():
    """One-instruction correction: out = y0 * q(g), g = (bits(y0)&mask)|1.0."""
    name = "EXP2_CORR_ANT"
    for op in dve_ops.OPS:
        if op.name == name:
            return op
    msk = Latch(Src1)
    a = Bin(AluOp.BITWISE_AND, Src0, msk)
    g = Bin(AluOp.BITWISE_OR, a, One)
    y = (((g * C0) + C1) * g + C2) * Src0

    def _ref(in0, in1, s0, s1, imm2):
        bits = in0.view(np.int32)
        g = ((bits & 0x007FFFFF) | 0x3F800000).view(np.float32)
        return (((g * s0 + s1) * g + imm2) * in0).astype(np.float32)

    spec = Spec(body=y, reference=_ref)
    ver = dve_ver_for("TRN2")
    row = dve_ops._CUSTOM_DVE_ROW_BASE + len(dve_ops.OPS)
    dve_ops._SUB_OPCODE_FOR_NAME[name] = row
    uops = lower(spec, ver=ver)
    sha = DveOpSpec(name=name, opcode=row, uops=uops, rd1_en=True).sha(ver)
    op = dve_ops.DveOp(name, spec, subdim=False, uops_sha={ver: sha})
    dve_ops.OPS.append(op)
    dve_ops.CUSTOM_DVE_SPECS[name] = spec
    return op


def build_attn(nc, tc, ctx: ExitStack, q, k, v, o, L, NH, LW=512, sc_bufs=2):
    VW = 65           # V columns + ones column
    ST = L // 128     # 128-row s tiles
    NCH = L // LW     # L windows per head
    NP = NH // 2      # head pairs

    exp_op = _register_exp2_corr()

    # q/k arrive E-major: [NH, 64, L] -> rows (head, e); v is l-major.
    q2 = q.rearrange("h e l -> (h e) l")
    k2 = k.rearrange("h e l -> (h e) l")
    vr = v.rearrange("(t p) h e -> p t h e", p=128)

    singles = ctx.enter_context(tc.tile_pool(name="singles", bufs=1))
    raw_pool = ctx.enter_context(tc.tile_pool(name="raw", bufs=2))
    tr_pool = ctx.enter_context(tc.tile_pool(name="tr", bufs=2))
    vp_pool = ctx.enter_context(tc.tile_pool(name="vp", bufs=4))
    pt_pool = ctx.enter_context(tc.tile_pool(name="pt", bufs=4))
    ti_pool = ctx.enter_context(tc.tile_pool(name="ti", bufs=3))
    sc_pool = ctx.enter_context(tc.tile_pool(name="sc", bufs=sc_bufs,
                                             space="PSUM"))
    pv_pool = ctx.enter_context(tc.tile_pool(name="pv", bufs=2, space="PSUM"))
    ep_pool = ctx.enter_context(tc.tile_pool(name="ep", bufs=3))
    out_pool = ctx.enter_context(tc.tile_pool(name="out", bufs=4))

    mask = singles.tile([128, 1], F32)
    nc.gpsimd.memset(mask, MASK_F32)

    jobs = [(hp, c) for hp in range(NP) for c in range(NCH)]
    NG = len(jobs) * ST

    state = {}
    sc_of, pt_of = {}, {}
    todo = {}  # loop-iteration -> [thunk]

    def defer(g, fn):
        todo.setdefault(g, []).append(fn)

    def emit_pair_loads(hp, n_split=None):
        # qt/kt as 4 chunk tiles [128, 512] bf16 (rows hi*64+e) so the first
        # QKs only depend on the first chunks; gpsimd cast-DMA from the
        # E-major DRAM layout.
        NCHK = L // LW
        qtc = [tr_pool.tile([128, LW], BF16, tag=f"qt{j}",
                            name=f"qt{hp}_{j}") for j in range(NCHK)]
        ktc = [tr_pool.tile([128, LW], BF16, tag=f"kt{j}",
                            name=f"kt{hp}_{j}") for j in range(NCHK)]
        r0 = 128 * hp
        order = [(qtc[0], q2, 0), (ktc[0], k2, 0)]
        order += [(ktc[j], k2, j) for j in range(1, NCHK)]
        order += [(qtc[j], q2, j) for j in range(1, NCHK)]
        for tile_, src_, j in order:
            nc.gpsimd.dma_start(out=tile_,
                                in_=src_[r0:r0 + 128, LW * j:LW * j + LW])
        rv = raw_pool.tile([128, ST, 2, 64], BF16, tag="rv", name=f"rv{hp}")
        nc.gpsimd.dma_start(out=rv, in_=vr[:, :, 2 * hp:2 * hp + 2, :])

        vps = []
        for hi in range(2):
            vp = vp_pool.tile([128, ST, VW], BF16, tag="vp",
                              name=f"vp{hp}_{hi}")
            nc.gpsimd.memset(vp[:, :, 64:65], 1.0)
            half = ST // 2
            for sp in range(2):
                ts0 = slice(half * sp, half * sp + half)
                nc.vector.tensor_copy(out=vp[:, ts0, 0:64],
                                      in_=rv[:, ts0, hi, :])
            vps.append(vp)
        state[hp] = (qtc, ktc, vps)

    def emit_qk(g):
        (hp, c), s = jobs[g // ST], g % ST
        if c == 0 and s == 0:
            if hp not in state:
                emit_pair_loads(hp)
            if hp + 1 < NP:
                emit_pair_loads(hp + 1)
        qtc, ktc, _ = state[hp]
        sc = sc_pool.tile([128, 2 * LW], F32, tag="sc", name=f"sc{g}")
        kj, ko = s // (LW // 128), 128 * (s % (LW // 128))
        for hi in range(2):
            nc.tensor.matmul(
                out=sc[:, LW * hi:LW * hi + LW],
                lhsT=ktc[kj][64 * hi:64 * hi + 64, ko:ko + 128],
                rhs=qtc[c][64 * hi:64 * hi + 64, :],
                start=True, stop=True, skip_group_check=True)
        sc_of[g] = sc

    def emit_exp(g):
        sc = sc_of.pop(g)
        pt = pt_pool.tile([128, 2 * LW], BF16, tag="pt", name=f"pt{g}")
        W = 2 * LW
        a0 = W - DVE_COLS
        nc.scalar.activation(out=pt[:, 0:a0], in_=sc[:, 0:a0], func=AF.Exp,
                             scale=SCALE)
        if DVE_COLS:
            ti = ti_pool.tile([128, DVE_COLS], I32, tag="ti", name=f"ti{g}")
            nc.vector.tensor_scalar(
                out=ti, in0=sc[:, a0:W],
                scalar1=A_CONST, scalar2=B_CONST,
                op0=ALU.mult, op1=ALU.add)
            nc.vector._custom_dve(
                exp_op, out=pt[:, a0:W], in0=ti.bitcast(F32), in1=mask,
                s0=_C2Q, s1=_C1Q, imm2=_C0Q)
        pt_of[g] = pt

    def emit_pv(g):
        (hp, c), s = jobs[g // ST], g % ST
        _, _, vps = state[hp]
        if s == 0:
            for hi in range(2):
                state[(hp, hi, c)] = pv_pool.tile(
                    [128, LW], F32, tag=f"pv{hi}", name=f"pv{g}_{hi}")
        pt = pt_of.pop(g)
        for hi in range(2):
            nc.tensor.matmul(
                out=state[(hp, hi, c)][0:VW, :],
                lhsT=vps[hi][:, s, :],
                rhs=pt[:, LW * hi:LW * hi + LW],
                start=(s == 0), stop=(s == ST - 1), skip_group_check=True)
        if s == ST - 1:
            for hi in range(2):
                emit_window_epilogue(g, hp, hi, c, state.pop((hp, hi, c)))

    def emit_window_epilogue(g, hp, hi, c, pv):
        # Evict promptly (frees the PSUM bank); the divide-by-sums chain is
        # spread over later steps so no engine FIFO blocks cross-engine.
        pvn = ep_pool.tile([64, LW], F32, tag="pvn")
        nc.vector.tensor_copy(out=pvn, in_=pv[0:64, :])
        sums0 = ep_pool.tile([1, LW], F32, tag="sums0")
        nc.vector.tensor_copy(out=sums0, in_=pv[64:65, :])
        rec = ep_pool.tile([1, LW], F32, tag="rec")
        rb = ep_pool.tile([64, LW], F32, tag="rb")
        if c == 0:
            state[("osb", hp, hi)] = out_pool.tile(
                [64, L], F32, tag="osb", name=f"osb{hp}_{hi}")
        osb = state[("osb", hp, hi)]
        # emit_pv(g) runs at loop iteration g+2; keys defer after that point.
        defer(g + 3 + hi,
              lambda: nc.vector.reciprocal_approx_fast(out=rec, in_=sums0))
        defer(g + 5 + hi, lambda: nc.gpsimd.partition_broadcast(rb, rec))
        defer(g + 7 + hi, lambda: nc.vector.tensor_tensor(
            out=osb[:, LW * c:LW * c + LW], in0=pvn, in1=rb, op=ALU.mult))
        if c == NCH - 1:
            defer(g + 9 + hi, lambda: nc.sync.dma_start(
                out=o[2 * hp + hi, :, :], in_=state.pop(("osb", hp, hi))))

    for g in range(NG + 12):
        for th in todo.pop(g, ()):
            th()
        if g < NG:
            emit_qk(g)
        if 1 <= g <= NG:
            emit_exp(g - 1)
        if 2 <= g < NG + 2:
            emit_pv(g - 2)
    for gg in sorted(todo):
        for th in todo.pop(gg):
            th()


def _build_program():
    nc = bacc.Bacc("TRN2", target_bir_lowering=False, debug=False,
                   num_devices=NCORES)
    q_t = nc.dram_tensor("q", [HPC, E_, L_], F32, kind="ExternalInput").ap()
    k_t = nc.dram_tensor("k", [HPC, E_, L_], F32, kind="ExternalInput").ap()
    v_t = nc.dram_tensor("v", [L_, HPC, E_], F32, kind="ExternalInput").ap()
    o_t = nc.dram_tensor("o", [HPC, E_, L_], F32, kind="ExternalOutput").ap()
    with tile.TileContext(nc) as tc:
        with ExitStack() as ctx:
            build_attn(nc, tc, ctx, q_t, k_t, v_t, o_t, L_, HPC)
    nc.compile()
    return nc


def kernel(queries, keys, values, attn_mask=None):
    """Full-problem entry: takes full [B,L,H,E] inputs, returns [B,L,H,D]."""
    global LAST_RESULTS, _PROG
    q = np.ascontiguousarray(np.asarray(queries, dtype=np.float32))
    k = np.ascontiguousarray(np.asarray(keys, dtype=np.float32))
    v = np.ascontiguousarray(np.asarray(values, dtype=np.float32))
    assert q.shape == (B_, L_, H_, E_), q.shape

    if _PROG is None:
        _PROG = _build_program()
    nc = _PROG

    in_maps = []
    for c in range(NCORES):
        b, h0 = c // 2, HPC * (c % 2)
        in_maps.append({
            # E-major per-core layout [head, e, l] for q/k (device needs no
            # input transposes); v stays l-major.
            "q": np.ascontiguousarray(q[b, :, h0:h0 + HPC, :].transpose(1, 2, 0)),
            "k": np.ascontiguousarray(k[b, :, h0:h0 + HPC, :].transpose(1, 2, 0)),
            "v": np.ascontiguousarray(v[b, :, h0:h0 + HPC, :]),
        })

    res = bass_utils.run_bass_kernel_spmd(nc, in_maps,
                                          core_ids=list(range(NCORES)))
    LAST_RESULTS = res

    out = np.empty((B_, L_, H_, E_), dtype=np.float32)
    for c in range(NCORES):
        b, h0 = c // 2, HPC * (c % 2)
        # device emits o[head, d, l]; undo the transpose host-side
        out[b, :, h0:h0 + HPC, :] = res.results[c]["o"].transpose(2, 0, 1)
    return out
